# revision 36
# baseline (speedup 1.0000x reference)
"""Chamfer loss kernel for Trainium2 (8 NeuronCores, SPMD data-parallel).

Problem: x, y of shape (2, 16, 1024, 3) fp32.
  dist[b, i, j] = sqrt(EPS + max(||x[b,j] - y[b,i]||^2, 0))  over (BT=32, N=1024, N)
  out = mean(min_i dist) + mean(min_j dist)   (a scalar)

Strategy:
  - Shard the fused BT=32 batch dim across 8 cores (4 batch elements/core).
  - Squared distances via a single K=5 augmented matmul on the PE array:
      sq[i,j] = [y2_i, 1, -2y_i0, -2y_i1, -2y_i2] . [1, x2_j, x_j0, x_j1, x_j2]
    (augmented operands are built on host; this is <0.5% of the FLOPs).
  - Both min-reductions become free-axis reductions by computing BOTH
    orientations of the distance matrix on the PE (cheap there, and it
    avoids partition-axis reductions entirely). Each 128x1024 PSUM block is
    reduced in one ~512-cycle DVE pass: ACT copies the first half to SBUF
    through its own PSUM port while a tensor_tensor_scan(min,min) streams
    the second half (PSUM port) against that copy (SBUF port); the scan's
    last column is the block's row minimum.
  - min(sqrt(eps+max(s,0))) == sqrt(eps+max(min(s),0)) by monotonicity, so
    only the (BT, N) min values need the sqrt/clamp, done on host along with
    the final mean (the "cheap all-reduce").
  - Matmul inputs are a 3-way bf16 split (hi/mid/lo) of the fp32 augmented
    operands, with the 6 significant cross-pairings stacked into K=30.
    bf16 products are exact in fp32 PSUM accumulation, so the result matches
    fp32 to ~2^-26 relative -- measured 5.6e-6 rel on the final scalar --
    while streaming at 1 cycle/row on the PE (plain fp32 is 4 cycles/row).
"""

import os
import sys

import numpy as np

for _p in ("/opt/trn_rl_repo",):
    if os.path.isdir(_p) and _p not in sys.path:
        sys.path.insert(0, _p)

import concourse.bass as bass
import concourse.mybir as mybir
from concourse.bass_utils import run_bass_kernel_spmd
from concourse.tile import TileContext, add_dep_helper


EPS = 1e-6
B, T, N, D = 2, 16, 1024, 3
BT = B * T
NCORES = 8
BPC = BT // NCORES  # batch elements per core
KAUG = 5  # augmented contraction dim
# (stationary-part, moving-part) bf16-split pairings; 0=hi 1=mid 2=lo.
# Dropped pairings are all < 2^-26 relative.
PAIRS = ((0, 0), (0, 1), (1, 0), (1, 1), (0, 2), (2, 0))
KSPLIT = KAUG * len(PAIRS)  # 30
NBLK = N // 128  # 128-row blocks per batch element
# PSUM-egress split per 1024-wide block: ACT copies the first X_A columns
# out through its PSUM port while the DVE min-scans the remaining X_D
# against them (the scan needs equal-length streams).
X_A = N // 2
X_D = N - X_A

# batch-major packed input: per batch element b, the four augmented
# operands at cols b*4*N + o*N for o in (y_st, x_mv, x_st, y_mv)
O_Y_ST, O_X_MV, O_X_ST, O_Y_MV = 0, 1, 2, 3
AUG_W = 4 * BPC * N

_F32 = mybir.dt.float32
_BF16 = mybir.dt.bfloat16

_NC_CACHE = None


def _build_nc():
    """One NeuronCore program; identical on all 8 cores (inputs differ)."""
    nc = bass.Bass()

    aug = nc.dram_tensor("aug", [KSPLIT, AUG_W], _BF16, kind="ExternalInput")
    # Per-point squared-distance minima: cols [0, BPC*NBLK) = per-y-point
    # (reduce over x), cols [BPC*NBLK, 2*BPC*NBLK) = per-x-point.
    # Col b*NBLK + blk holds points blk*128 + p of batch element b.
    mins = nc.dram_tensor("mins", [128, 2 * BPC * NBLK], _F32, kind="ExternalOutput")

    with TileContext(nc) as tc:
        with (
            tc.tile_pool(name="inp", bufs=1) as inp_pool,
            tc.tile_pool(name="outp", bufs=1) as out_pool,
            tc.tile_pool(name="scr", bufs=1) as scr_pool,
            tc.tile_pool(name="ps", bufs=4, space="PSUM") as ps_pool,
        ):
            aug_t = inp_pool.tile([KSPLIT, AUG_W], _BF16, name="aug_t")
            # PE warm-up: ~3.4us of dummy matmuls overlapping the input DMA
            # so the HAM clock gate reaches full rate before real work
            warm_t = scr_pool.tile([32, 640], _BF16, name="warm_t")
            nc.gpsimd.memset(warm_t[:], 0.0)
            for w in range(4):
                wps = ps_pool.tile([128, N], _F32, tag="ps")
                for h in range(2):
                    nc.tensor.matmul(
                        wps[:, h * 512 : (h + 1) * 512],
                        warm_t[:, 0:128],
                        warm_t[:, 128:640],
                        start=True,
                        stop=True,
                    )
            # chunked load (one chunk per half batch element) so compute on
            # batch b overlaps the remaining batches' input transfer
            CHW = AUG_W // 8
            for c in range(8):
                nc.sync.dma_start(
                    out=aug_t[:, c * CHW : (c + 1) * CHW],
                    in_=aug[:, c * CHW : (c + 1) * CHW],
                )

            mins_t = out_pool.tile([128, 2 * BPC * NBLK], _F32, name="mins_t")
            # scan-output rings (double-buffered per 8-block group); only
            # each 512-wide lane's last column (the block min) is live,
            # extracted by one strided ACT copy per group. The extract for
            # group g is emitted two tiles into group g+1 so it never sits
            # ahead of that group's PSUM half-copies in the ACT queue.
            rings = [
                scr_pool.tile([128, NBLK * X_D], _F32, name=f"ring{i}_t")
                for i in range(2)
            ]
            pending_extract = []  # [(ring, gcol)]

            for b in range(BPC):
                # orientation 0: rows = y points (reduce over x)
                # orientation 1: rows = x points (reduce over y)
                for half, (st_o, mv_o) in enumerate(
                    ((O_Y_ST, O_X_MV), (O_X_ST, O_Y_MV))
                ):
                    st_off = (b * 4 + st_o) * N
                    mv_off = (b * 4 + mv_o) * N
                    for blk in range(NBLK):
                        ps = ps_pool.tile([128, N], _F32, tag="ps")
                        lo = st_off + blk * 128
                        lhsT = aug_t[:, lo : lo + 128]
                        mm = None
                        for h in range(N // 512):
                            ro = mv_off + h * 512
                            mm = nc.tensor.matmul(
                                ps[:, h * 512 : (h + 1) * 512],
                                lhsT,
                                aug_t[:, ro : ro + 512],
                                start=True,
                                stop=True,
                            )
                        # Split the PSUM egress across both PSUM read ports:
                        # ACT copies the first 512-half to SBUF while DVE
                        # min-scans the second half (PSUM port) against that
                        # copy (SBUF port) — one ~512-cycle DVE pass per
                        # 1024-wide block; the scan's last column is the
                        # block's row minimum.
                        h0 = scr_pool.tile([128, X_A], _F32, tag="h0", bufs=12)
                        cp = nc.scalar.copy(h0[:], ps[:, 0:X_A])
                        # let the copy's PE wait cover the tile's second
                        # matmul too, so the scan's PE dep prunes transitively
                        add_dep_helper(cp.ins, mm.ins, sync=True,
                                       reason="cp waits full psum tile")
                        if blk == 2 and pending_extract:
                            pr, pg = pending_extract.pop()
                            nc.scalar.copy(
                                mins_t[:, pg : pg + NBLK],
                                pr[:, X_D - 1 :: X_D],
                            )
                        ring_t = rings[(b * 2 + half) % 2]
                        lane = ring_t[:, blk * X_D : (blk + 1) * X_D]
                        nc.vector.tensor_tensor_scan(
                            lane,
                            ps[:, X_A:N],
                            h0[:, 0:X_D],
                            3.0e38,
                            mybir.AluOpType.min,
                            mybir.AluOpType.min,
                        )
                    gcol = half * BPC * NBLK + b * NBLK
                    pending_extract.append((ring_t, gcol))

            for pr, pg in pending_extract:
                nc.scalar.copy(
                    mins_t[:, pg : pg + NBLK], pr[:, X_D - 1 :: X_D]
                )
            nc.sync.dma_start(out=mins[:], in_=mins_t[:])


    return nc


def _strip_redundant_waits(nc):
    """Transitive vector-clock closure over the emitted sync graph; drops
    every semaphore wait whose condition is already implied at the waiting
    instruction's dispatch point.

    Soundness: semaphores only increase; each engine/DMA queue dispatches and
    completes its instructions in program order (PE completion is pc-monotone
    per the HW docs; DVE/ACT are serial with a pipeline drain between ops).
    So (a) an instruction inherits everything instructions earlier on its own
    engine acquired via their waits, and for serial engines also everything
    published by their completions, and (b) waiting `sem >= v` also conveys
    the dispatch-knowledge of the instruction whose completion brought `sem`
    to `v` (plus, by in-order completion, of all earlier instructions on that
    engine). Tile emits waits per-processor without this closure, which
    overflows the per-opcode sync-wait encoding budget (walrus "Too many
    sync wait commands")."""

    def merge(dst, srcd):
        for s, v in srcd.items():
            if dst.get(s, -1) < v:
                dst[s] = v

    # The emitted program is straight-line (python-unrolled, no hardware
    # loops), so sync state flows across basic blocks in order.
    cum = {}  # sem id -> cumulative inc value so far
    poisoned = set()  # sems with non-inc updates: no pruning
    publishes = {}  # sem id -> list of (value, knowledge dict), ascending
    know = {}  # engine -> dispatch knowledge {sem: value}
    done_know = {}  # engine -> completion knowledge union of all its insts
    for bb in nc.m.functions[0].blocks:
        for inst in bb.instructions:
            si = inst.sync_info
            if si is None:
                continue
            e = inst.engine
            k = know.setdefault(e, {})
            dk = done_know.setdefault(e, {})
            if e in (mybir.EngineType.DVE, mybir.EngineType.Activation):
                # serial engines dispatch only after the prior op completed
                merge(k, dk)
            ws = si.on_wait or []

            def absorbed(base, waits):
                kk = dict(base)
                for w2 in waits:
                    v2 = w2.wait_value or 0
                    if kk.get(w2.id, -1) < v2:
                        kk[w2.id] = v2
                    for pv, pk in publishes.get(w2.id, ()):
                        if pv <= kk.get(w2.id, -1):
                            merge(kk, pk)
                return kk

            prunable = [
                w
                for w in ws
                if w.sync_type == "semaphore"
                and w.wait_mode == "sem-ge-imm"
                and w.wait_reg is None
                and w.id not in poisoned
            ]
            fixed = [w for w in ws if w not in prunable]
            # fixpoint: drop any wait implied by engine knowledge plus the
            # transitive knowledge of the remaining waits
            kept = list(prunable)
            changed = True
            while changed:
                changed = False
                for w in list(kept):
                    others = [x for x in kept if x is not w] + fixed
                    if absorbed(k, others).get(w.id, -1) >= (w.wait_value or 0):
                        kept.remove(w)
                        changed = True
            # knowledge gained includes even dropped waits (they were implied)
            k.update(absorbed(k, ws))
            if len(kept) + len(fixed) != len(ws):
                si.on_wait = fixed + kept
            # completion: publish knowledge at each inc
            ups = [
                u
                for u in (si.on_update or [])
                if u.sync_type == "semaphore"
            ]
            bad = [u for u in ups if u.update_mode not in ("sem-inc", "sem-add-imm")]
            for u in bad:
                poisoned.add(u.id)
                publishes.pop(u.id, None)
            ups = [u for u in ups if u.update_mode in ("sem-inc", "sem-add-imm")]
            if ups:
                snap = dict(dk)
                merge(snap, k)
                for u in ups:
                    cum[u.id] = cum.get(u.id, 0) + (u.update_value or 0)
                for u in ups:
                    snap[u.id] = max(snap.get(u.id, -1), cum[u.id])
                for u in ups:
                    if u.id not in poisoned:
                        publishes.setdefault(u.id, []).append((cum[u.id], snap))
                merge(dk, snap)
            else:
                merge(dk, k)
    return nc


def _get_nc():
    global _NC_CACHE
    if _NC_CACHE is None:
        _NC_CACHE = _strip_redundant_waits(_build_nc())
    return _NC_CACHE


def _augment(seg):
    """seg: (BPC, N, 3) -> moving (5, BPC*N) = [1, p2, p] and
    stationary (5, BPC*N) = [p2, 1, -2p]."""
    p2 = np.einsum("bnd,bnd->bn", seg, seg).astype(np.float32)  # (BPC, N)
    coords = seg.transpose(2, 0, 1)  # (3, BPC, N)
    mv = np.empty((KAUG, BPC, N), dtype=np.float32)
    mv[0] = 1.0
    mv[1] = p2
    mv[2:5] = coords
    st = np.empty((KAUG, BPC, N), dtype=np.float32)
    st[0] = p2
    st[1] = 1.0
    st[2:5] = -2.0 * coords
    return mv.reshape(KAUG, BPC * N), st.reshape(KAUG, BPC * N)


try:
    from ml_dtypes import bfloat16 as _np_bf16
except ImportError:  # jax always ships ml_dtypes, but keep a clear error
    raise RuntimeError("ml_dtypes required for bf16 host-side splitting")


def _split3(a):
    """fp32 -> (hi, mid, lo) bf16 triple with hi+mid+lo == a to ~2^-26 rel."""
    hi = a.astype(_np_bf16)
    rem = a - hi.astype(np.float32)
    mid = rem.astype(_np_bf16)
    lo = (rem - mid.astype(np.float32)).astype(_np_bf16)
    return hi, mid, lo


def _stack_pairs(parts, which):
    """parts: (hi, mid, lo) of a (KAUG, W) operand; stack the split parts for
    each pairing (PAIRS[g][which]) into a (KSPLIT, W) bf16 array."""
    return np.concatenate([parts[p[which]] for p in PAIRS], axis=0)


def _unshard_min(col_tile):
    """(128, BPC*NBLK) device layout -> (BPC, N) with point = blk*128 + p."""
    return col_tile.reshape(128, BPC, NBLK).transpose(1, 2, 0).reshape(BPC, N)


def _run_device(x, y, trace=False, **kw):
    xf = np.asarray(x, dtype=np.float32).reshape(BT, N, D)
    yf = np.asarray(y, dtype=np.float32).reshape(BT, N, D)

    in_maps = []
    for c in range(NCORES):
        x_mv, x_st = _augment(xf[c * BPC : (c + 1) * BPC])
        y_mv, y_st = _augment(yf[c * BPC : (c + 1) * BPC])
        ops = (
            _stack_pairs(_split3(y_st), 0),
            _stack_pairs(_split3(x_mv), 1),
            _stack_pairs(_split3(x_st), 0),
            _stack_pairs(_split3(y_mv), 1),
        )
        # batch-major: (KSPLIT, BPC, 4, N)
        aug = np.stack(
            [o.reshape(KSPLIT, BPC, N) for o in ops], axis=2
        ).reshape(KSPLIT, AUG_W)
        in_maps.append({"aug": np.ascontiguousarray(aug)})

    res = run_bass_kernel_spmd(
        _get_nc(), in_maps, list(range(NCORES)), trace=trace, **kw
    )

    half = BPC * NBLK
    min_y = np.concatenate(
        [_unshard_min(res.results[c]["mins"][:, :half]) for c in range(NCORES)]
    )  # (BT, N) per-y-point min squared distance
    min_x = np.concatenate(
        [_unshard_min(res.results[c]["mins"][:, half:]) for c in range(NCORES)]
    )
    return min_x, min_y, res


def kernel(x, y):
    min_x, min_y, _ = _run_device(x, y)
    d_x = np.sqrt(EPS + np.maximum(min_x, 0.0), dtype=np.float32)
    d_y = np.sqrt(EPS + np.maximum(min_y, 0.0), dtype=np.float32)
    out = d_x.mean(dtype=np.float32) + d_y.mean(dtype=np.float32)
    return np.asarray(out, dtype=np.float32)



# revision 42
# speedup vs baseline: 1.0330x; 1.0330x over previous
"""Chamfer loss kernel for Trainium2 (8 NeuronCores, SPMD data-parallel).

Problem: x, y of shape (2, 16, 1024, 3) fp32.
  dist[b, i, j] = sqrt(EPS + max(||x[b,j] - y[b,i]||^2, 0))  over (BT=32, N=1024, N)
  out = mean(min_i dist) + mean(min_j dist)   (a scalar)

Strategy:
  - Shard the fused BT=32 batch dim across 8 cores (4 batch elements/core).
  - Squared distances via a single K=5 augmented matmul on the PE array:
      sq[i,j] = [y2_i, 1, -2y_i0, -2y_i1, -2y_i2] . [1, x2_j, x_j0, x_j1, x_j2]
    (augmented operands are built on host; this is <0.5% of the FLOPs).
  - Both min-reductions become free-axis reductions by computing BOTH
    orientations of the distance matrix on the PE (cheap there, and it
    avoids partition-axis reductions entirely). Each 128x1024 PSUM block is
    reduced in one ~512-cycle DVE pass: ACT copies the first half to SBUF
    through its own PSUM port while a tensor_tensor_scan(min,min) streams
    the second half (PSUM port) against that copy (SBUF port); the scan's
    last column is the block's row minimum.
  - min(sqrt(eps+max(s,0))) == sqrt(eps+max(min(s),0)) by monotonicity, so
    only the (BT, N) min values need the sqrt/clamp, done on host along with
    the final mean (the "cheap all-reduce").
  - Matmul inputs are a 3-way bf16 split (hi/mid/lo) of the fp32 augmented
    operands, with the 6 significant cross-pairings stacked into K=30.
    bf16 products are exact in fp32 PSUM accumulation, so the result matches
    fp32 to ~2^-26 relative -- measured 5.6e-6 rel on the final scalar --
    while streaming at 1 cycle/row on the PE (plain fp32 is 4 cycles/row).
"""

import os
import sys

import numpy as np

for _p in ("/opt/trn_rl_repo",):
    if os.path.isdir(_p) and _p not in sys.path:
        sys.path.insert(0, _p)

import concourse.bass as bass
import concourse.mybir as mybir
from concourse.bass_utils import run_bass_kernel_spmd
from concourse.tile import TileContext, add_dep_helper


EPS = 1e-6
B, T, N, D = 2, 16, 1024, 3
BT = B * T
NCORES = 8
BPC = BT // NCORES  # batch elements per core
KAUG = 5  # augmented contraction dim
# (stationary-part, moving-part) bf16-split pairings; 0=hi 1=mid 2=lo.
# Dropped pairings are all < 2^-26 relative.
PAIRS = ((0, 0), (0, 1), (1, 0), (1, 1), (0, 2), (2, 0))
KSPLIT = KAUG * len(PAIRS)  # 30
NBLK = N // 128  # 128-row blocks per batch element
# PSUM-egress split per 1024-wide block: ACT copies the first X_A columns
# out through its PSUM port while the DVE min-scans the remaining X_D
# against them (the scan needs equal-length streams).
X_A = N // 2
X_D = N - X_A

# batch-major packed input: per batch element b, the four augmented
# operands at cols b*4*N + o*N for o in (y_st, x_mv, x_st, y_mv)
O_Y_ST, O_X_MV, O_X_ST, O_Y_MV = 0, 1, 2, 3
AUG_W = 4 * BPC * N

_F32 = mybir.dt.float32
_BF16 = mybir.dt.bfloat16

_NC_CACHE = None


def _build_nc():
    """One NeuronCore program; identical on all 8 cores (inputs differ)."""
    nc = bass.Bass()

    aug = nc.dram_tensor("aug", [KSPLIT, AUG_W], _BF16, kind="ExternalInput")
    # Per-point squared-distance minima: cols [0, BPC*NBLK) = per-y-point
    # (reduce over x), cols [BPC*NBLK, 2*BPC*NBLK) = per-x-point.
    # Col b*NBLK + blk holds points blk*128 + p of batch element b.
    mins = nc.dram_tensor("mins", [128, 2 * BPC * NBLK], _F32, kind="ExternalOutput")

    with TileContext(nc) as tc:
        with (
            tc.tile_pool(name="inp", bufs=1) as inp_pool,
            tc.tile_pool(name="outp", bufs=1) as out_pool,
            tc.tile_pool(name="scr", bufs=1) as scr_pool,
            tc.tile_pool(name="ps", bufs=4, space="PSUM") as ps_pool,
        ):
            aug_t = inp_pool.tile([KSPLIT, AUG_W], _BF16, name="aug_t")
            # PE warm-up: ~3.4us of dummy matmuls overlapping the input DMA
            # so the HAM clock gate reaches full rate before real work
            warm_t = scr_pool.tile([32, 640], _BF16, name="warm_t")
            nc.gpsimd.memset(warm_t[:], 0.0)
            for w in range(2):
                wps = ps_pool.tile([128, N], _F32, tag="ps")
                for h in range(2):
                    nc.tensor.matmul(
                        wps[:, h * 512 : (h + 1) * 512],
                        warm_t[:, 0:128],
                        warm_t[:, 128:640],
                        start=True,
                        stop=True,
                    )
            # chunked load (one chunk per half batch element) so compute on
            # batch b overlaps the remaining batches' input transfer
            CHW = AUG_W // 8
            for c in range(8):
                nc.sync.dma_start(
                    out=aug_t[:, c * CHW : (c + 1) * CHW],
                    in_=aug[:, c * CHW : (c + 1) * CHW],
                )

            mins_t = out_pool.tile([128, 2 * BPC * NBLK], _F32, name="mins_t")
            # scan-output rings (double-buffered per 8-block group); only
            # each 512-wide lane's last column (the block min) is live,
            # extracted by one strided ACT copy per group. The extract for
            # group g is emitted two tiles into group g+1 so it never sits
            # ahead of that group's PSUM half-copies in the ACT queue.
            rings = [
                scr_pool.tile([128, NBLK * X_D], _F32, name=f"ring{i}_t")
                for i in range(2)
            ]
            pending_extract = []  # [(ring, gcol)]

            for b in range(BPC):
                # orientation 0: rows = y points (reduce over x)
                # orientation 1: rows = x points (reduce over y)
                for half, (st_o, mv_o) in enumerate(
                    ((O_Y_ST, O_X_MV), (O_X_ST, O_Y_MV))
                ):
                    st_off = (b * 4 + st_o) * N
                    mv_off = (b * 4 + mv_o) * N
                    for blk in range(NBLK):
                        ps = ps_pool.tile([128, N], _F32, tag="ps")
                        lo = st_off + blk * 128
                        lhsT = aug_t[:, lo : lo + 128]
                        mm = None
                        for h in range(N // 512):
                            ro = mv_off + h * 512
                            mm = nc.tensor.matmul(
                                ps[:, h * 512 : (h + 1) * 512],
                                lhsT,
                                aug_t[:, ro : ro + 512],
                                start=True,
                                stop=True,
                            )
                        # Split the PSUM egress across both PSUM read ports:
                        # ACT copies the first 512-half to SBUF while DVE
                        # min-scans the second half (PSUM port) against that
                        # copy (SBUF port) — one ~512-cycle DVE pass per
                        # 1024-wide block; the scan's last column is the
                        # block's row minimum.
                        h0 = scr_pool.tile([128, X_A], _F32, tag="h0", bufs=12)
                        cp = nc.scalar.copy(h0[:], ps[:, 0:X_A])
                        # let the copy's PE wait cover the tile's second
                        # matmul too, so the scan's PE dep prunes transitively
                        add_dep_helper(cp.ins, mm.ins, sync=True,
                                       reason="cp waits full psum tile")
                        if blk == 2 and pending_extract:
                            pr, pg = pending_extract.pop()
                            nc.scalar.copy(
                                mins_t[:, pg : pg + NBLK],
                                pr[:, X_D - 1 :: X_D],
                            )
                            nc.sync.dma_start(
                                out=mins[:, pg : pg + NBLK],
                                in_=mins_t[:, pg : pg + NBLK],
                            )
                        ring_t = rings[(b * 2 + half) % 2]
                        lane = ring_t[:, blk * X_D : (blk + 1) * X_D]
                        nc.vector.tensor_tensor_scan(
                            lane,
                            ps[:, X_A:N],
                            h0[:, 0:X_D],
                            3.0e38,
                            mybir.AluOpType.min,
                            mybir.AluOpType.min,
                        )
                    gcol = half * BPC * NBLK + b * NBLK
                    pending_extract.append((ring_t, gcol))

            for pr, pg in pending_extract:
                nc.scalar.copy(
                    mins_t[:, pg : pg + NBLK], pr[:, X_D - 1 :: X_D]
                )
                nc.sync.dma_start(
                    out=mins[:, pg : pg + NBLK],
                    in_=mins_t[:, pg : pg + NBLK],
                )


    return nc


def _strip_redundant_waits(nc):
    """Transitive vector-clock closure over the emitted sync graph; drops
    every semaphore wait whose condition is already implied at the waiting
    instruction's dispatch point.

    Soundness: semaphores only increase; each engine/DMA queue dispatches and
    completes its instructions in program order (PE completion is pc-monotone
    per the HW docs; DVE/ACT are serial with a pipeline drain between ops).
    So (a) an instruction inherits everything instructions earlier on its own
    engine acquired via their waits, and for serial engines also everything
    published by their completions, and (b) waiting `sem >= v` also conveys
    the dispatch-knowledge of the instruction whose completion brought `sem`
    to `v` (plus, by in-order completion, of all earlier instructions on that
    engine). Tile emits waits per-processor without this closure, which
    overflows the per-opcode sync-wait encoding budget (walrus "Too many
    sync wait commands")."""

    def merge(dst, srcd):
        for s, v in srcd.items():
            if dst.get(s, -1) < v:
                dst[s] = v

    # The emitted program is straight-line (python-unrolled, no hardware
    # loops), so sync state flows across basic blocks in order.
    cum = {}  # sem id -> cumulative inc value so far
    poisoned = set()  # sems with non-inc updates: no pruning
    publishes = {}  # sem id -> list of (value, knowledge dict), ascending
    know = {}  # engine -> dispatch knowledge {sem: value}
    done_know = {}  # engine -> completion knowledge union of all its insts
    for bb in nc.m.functions[0].blocks:
        for inst in bb.instructions:
            si = inst.sync_info
            if si is None:
                continue
            e = inst.engine
            k = know.setdefault(e, {})
            dk = done_know.setdefault(e, {})
            if e in (mybir.EngineType.DVE, mybir.EngineType.Activation):
                # serial engines dispatch only after the prior op completed
                merge(k, dk)
            ws = si.on_wait or []

            def absorbed(base, waits):
                kk = dict(base)
                for w2 in waits:
                    v2 = w2.wait_value or 0
                    if kk.get(w2.id, -1) < v2:
                        kk[w2.id] = v2
                    for pv, pk in publishes.get(w2.id, ()):
                        if pv <= kk.get(w2.id, -1):
                            merge(kk, pk)
                return kk

            prunable = [
                w
                for w in ws
                if w.sync_type == "semaphore"
                and w.wait_mode == "sem-ge-imm"
                and w.wait_reg is None
                and w.id not in poisoned
            ]
            fixed = [w for w in ws if w not in prunable]
            # fixpoint: drop any wait implied by engine knowledge plus the
            # transitive knowledge of the remaining waits
            kept = list(prunable)
            changed = True
            while changed:
                changed = False
                for w in list(kept):
                    others = [x for x in kept if x is not w] + fixed
                    if absorbed(k, others).get(w.id, -1) >= (w.wait_value or 0):
                        kept.remove(w)
                        changed = True
            # knowledge gained includes even dropped waits (they were implied)
            k.update(absorbed(k, ws))
            if len(kept) + len(fixed) != len(ws):
                si.on_wait = fixed + kept
            # completion: publish knowledge at each inc
            ups = [
                u
                for u in (si.on_update or [])
                if u.sync_type == "semaphore"
            ]
            bad = [u for u in ups if u.update_mode not in ("sem-inc", "sem-add-imm")]
            for u in bad:
                poisoned.add(u.id)
                publishes.pop(u.id, None)
            ups = [u for u in ups if u.update_mode in ("sem-inc", "sem-add-imm")]
            if ups:
                snap = dict(dk)
                merge(snap, k)
                for u in ups:
                    cum[u.id] = cum.get(u.id, 0) + (u.update_value or 0)
                for u in ups:
                    snap[u.id] = max(snap.get(u.id, -1), cum[u.id])
                for u in ups:
                    if u.id not in poisoned:
                        publishes.setdefault(u.id, []).append((cum[u.id], snap))
                merge(dk, snap)
            else:
                merge(dk, k)
    return nc


def _get_nc():
    global _NC_CACHE
    if _NC_CACHE is None:
        _NC_CACHE = _strip_redundant_waits(_build_nc())
    return _NC_CACHE


def _augment(seg):
    """seg: (BPC, N, 3) -> moving (5, BPC*N) = [1, p2, p] and
    stationary (5, BPC*N) = [p2, 1, -2p]."""
    p2 = np.einsum("bnd,bnd->bn", seg, seg).astype(np.float32)  # (BPC, N)
    coords = seg.transpose(2, 0, 1)  # (3, BPC, N)
    mv = np.empty((KAUG, BPC, N), dtype=np.float32)
    mv[0] = 1.0
    mv[1] = p2
    mv[2:5] = coords
    st = np.empty((KAUG, BPC, N), dtype=np.float32)
    st[0] = p2
    st[1] = 1.0
    st[2:5] = -2.0 * coords
    return mv.reshape(KAUG, BPC * N), st.reshape(KAUG, BPC * N)


try:
    from ml_dtypes import bfloat16 as _np_bf16
except ImportError:  # jax always ships ml_dtypes, but keep a clear error
    raise RuntimeError("ml_dtypes required for bf16 host-side splitting")


def _split3(a):
    """fp32 -> (hi, mid, lo) bf16 triple with hi+mid+lo == a to ~2^-26 rel."""
    hi = a.astype(_np_bf16)
    rem = a - hi.astype(np.float32)
    mid = rem.astype(_np_bf16)
    lo = (rem - mid.astype(np.float32)).astype(_np_bf16)
    return hi, mid, lo


def _stack_pairs(parts, which):
    """parts: (hi, mid, lo) of a (KAUG, W) operand; stack the split parts for
    each pairing (PAIRS[g][which]) into a (KSPLIT, W) bf16 array."""
    return np.concatenate([parts[p[which]] for p in PAIRS], axis=0)


def _unshard_min(col_tile):
    """(128, BPC*NBLK) device layout -> (BPC, N) with point = blk*128 + p."""
    return col_tile.reshape(128, BPC, NBLK).transpose(1, 2, 0).reshape(BPC, N)


def _run_device(x, y, trace=False, **kw):
    xf = np.asarray(x, dtype=np.float32).reshape(BT, N, D)
    yf = np.asarray(y, dtype=np.float32).reshape(BT, N, D)

    in_maps = []
    for c in range(NCORES):
        x_mv, x_st = _augment(xf[c * BPC : (c + 1) * BPC])
        y_mv, y_st = _augment(yf[c * BPC : (c + 1) * BPC])
        ops = (
            _stack_pairs(_split3(y_st), 0),
            _stack_pairs(_split3(x_mv), 1),
            _stack_pairs(_split3(x_st), 0),
            _stack_pairs(_split3(y_mv), 1),
        )
        # batch-major: (KSPLIT, BPC, 4, N)
        aug = np.stack(
            [o.reshape(KSPLIT, BPC, N) for o in ops], axis=2
        ).reshape(KSPLIT, AUG_W)
        in_maps.append({"aug": np.ascontiguousarray(aug)})

    res = run_bass_kernel_spmd(
        _get_nc(), in_maps, list(range(NCORES)), trace=trace, **kw
    )

    half = BPC * NBLK
    min_y = np.concatenate(
        [_unshard_min(res.results[c]["mins"][:, :half]) for c in range(NCORES)]
    )  # (BT, N) per-y-point min squared distance
    min_x = np.concatenate(
        [_unshard_min(res.results[c]["mins"][:, half:]) for c in range(NCORES)]
    )
    return min_x, min_y, res


def kernel(x, y):
    min_x, min_y, _ = _run_device(x, y)
    d_x = np.sqrt(EPS + np.maximum(min_x, 0.0), dtype=np.float32)
    d_y = np.sqrt(EPS + np.maximum(min_y, 0.0), dtype=np.float32)
    out = d_x.mean(dtype=np.float32) + d_y.mean(dtype=np.float32)
    return np.asarray(out, dtype=np.float32)

